# revision 27
# baseline (speedup 1.0000x reference)
"""Multi-head attention (B=2, S=2048, D=768, H=12) on 8 trn2 NeuronCores.

Sharding: batch x head-group data/tensor parallel. Core c = b*4+g handles
batch b and heads [3g, 3g+3) (a 192-wide slice of the QKV projections and
the matching 192-row slice of Wo). Each core emits a partial [2048, 768]
output; the host sums the 4 head-group partials per batch and adds bo.

Device layout notes:
- Inputs are transposed on host to [d_model, seq] and cast to fp16 so the
  TensorEngine (which contracts over the partition dim) can consume them
  directly; all matmuls run on fp16 operands with fp32 PSUM accumulation.
- Attention works on transposed scores sT[k, q] so softmax's sum over k
  becomes a matmul reduction: v is augmented with a ones column, so the
  ctx matmul yields both ctx^T and the softmax denominator in one pass.
  exp() needs no max-subtraction: |scores/8| <= ~11 for this problem.
- Normalization multiplies ctx^T by 1/denom broadcast across partitions
  (GPSIMD partition_broadcast), then the output projection runs from
  ctx^T directly.
- Heads 0/1 live at SBUF partitions 0-63/64-127 so their score matmuls
  land in different PE row groups and overlap; head 2's operands are
  mirrored into both halves for the same reason.
- The output projection for q-tile j is emitted after q-tile j+1's
  attention so the PE stream never stalls on the normalize chain.
"""

import numpy as np

D_MODEL = 768
NUM_HEADS = 12
D_K = 64
B = 2
S = 2048
N_CORES = 8
G = 4              # head groups (cores per batch)
GW = D_MODEL // G  # 192 features per group = 3 heads
HPG = 3            # heads per group
DC = D_MODEL // 128  # 6 d_model chunks
QT = 512           # q-tile width
NQT = S // QT      # 4
KC = S // 128      # 16 k chunks
ST = S // 128      # 16 seq tiles
WPK = 3 * DC * GW + 2 * D_MODEL  # packed weights columns: 4992
BPK = 8            # packed bias columns

_PROGRAM = None


def _build_program():
    from concourse import bacc, tile
    import concourse.mybir as mybir

    f16 = mybir.dt.float16
    f32 = mybir.dt.float32
    Exp = mybir.ActivationFunctionType.Exp
    mult = mybir.AluOpType.mult

    nc = bacc.Bacc("TRN2", target_bir_lowering=False, debug=False,
                   enable_asserts=False)

    xqT = nc.dram_tensor("xqT", [D_MODEL, S], f16, kind="ExternalInput")
    xkT = nc.dram_tensor("xkT", [D_MODEL, S], f16, kind="ExternalInput")
    xvT = nc.dram_tensor("xvT", [D_MODEL, S], f16, kind="ExternalInput")
    wpk = nc.dram_tensor("wpk", [128, WPK], f16, kind="ExternalInput")
    bpk = nc.dram_tensor("bpk", [128, BPK], f32, kind="ExternalInput")
    out = nc.dram_tensor("out", [S, D_MODEL], f32, kind="ExternalOutput")

    with tile.TileContext(nc) as tc:
        with tc.tile_pool(name="const", bufs=1) as cp, \
             tc.tile_pool(name="expp", bufs=6) as ep, \
             tc.tile_pool(name="normp", bufs=2) as np_, \
             tc.tile_pool(name="outp", bufs=2) as op, \
             tc.tile_pool(name="ps_s", bufs=2, space="PSUM") as ps_s, \
             tc.tile_pool(name="ps_c", bufs=3, space="PSUM") as ps_c, \
             tc.tile_pool(name="ps_o", bufs=1, space="PSUM") as ps_o:

            # ---- packed weights + biases. wk occupies the first
            # columns and ships in its own DMA so the k-projection can
            # start as soon as possible ----
            wps = cp.tile([128, WPK], f16, name="wps")
            nc.sync.dma_start(out=wps[:, 0:DC * GW], in_=wpk[:, 0:DC * GW])
            bps = cp.tile([128, BPK], f32, name="bps")
            nc.sync.dma_start(out=bps[:], in_=bpk[:])
            wk_sb = [wps[:, d * GW:(d + 1) * GW] for d in range(DC)]
            wq_sb = [wps[:, DC * GW + d * GW:DC * GW + (d + 1) * GW]
                     for d in range(DC)]
            wv_sb = [wps[:, 2 * DC * GW + d * GW:2 * DC * GW + (d + 1) * GW]
                     for d in range(DC)]
            wo_a = wps[:, 3 * DC * GW:3 * DC * GW + D_MODEL]
            wo_b = wps[0:64, 3 * DC * GW + D_MODEL:WPK]
            bq_a, bq_b = bps[:, 0:1], bps[0:64, 1:2]
            bk_a, bk_b = bps[:, 2:3], bps[0:64, 3:4]
            bv_h = [bps[0:64, 4 + h:5 + h] for h in range(HPG)]

            # ---- inputs: k first, then v, then q (attention needs full
            # kT and v before it can start, q only per-tile) ----
            xq_sb, xk_sb, xv_sb = [], [], []
            for d in range(DC):
                t = cp.tile([128, S], f16, name=f"xk{d}")
                nc.sync.dma_start(out=t[:], in_=xkT[d * 128:(d + 1) * 128, :])
                xk_sb.append(t)
            # remaining weights (wq/wv/wo) ship after xk so the k
            # projection's data isn't stuck behind them in the DMA queue
            nc.sync.dma_start(out=wps[:, DC * GW:WPK],
                              in_=wpk[:, DC * GW:WPK])
            for d in range(DC):
                t = cp.tile([128, S], f16, name=f"xq{d}")
                nc.sync.dma_start(out=t[:], in_=xqT[d * 128:(d + 1) * 128, :])
                xq_sb.append(t)
            for d in range(DC):
                t = cp.tile([128, S], f16, name=f"xv{d}")
                nc.sync.dma_start(out=t[:], in_=xvT[d * 128:(d + 1) * 128, :])
                xv_sb.append(t)

            # ---- projections. Order: kT, v, qT (dependency order) ----
            qT_a = cp.tile([128, S], f16, name="qT_a")
            qT_b = cp.tile([128, S], f16, name="qT_b")
            kT_a = cp.tile([128, S], f16, name="kT_a")
            kT_b = cp.tile([128, S], f16, name="kT_b")

            def proj_passA(x_sb, w_sb, b_a, dst_a):
                # features 0:128 (heads 0+1), d-outer accumulation: each
                # input chunk is consumed as it arrives from HBM. The
                # attention S-pool is idle here, so borrow its slots.
                pj = [ps_s.tile([128, 2 * QT], f32, name="S", tag="s")
                      for _ in range(2)]
                for d in range(DC):
                    for j2 in range(2):
                        for n in range(2):
                            cs = slice(j2 * 1024 + n * QT,
                                       j2 * 1024 + (n + 1) * QT)
                            nc.tensor.matmul(
                                pj[j2][:, n * QT:(n + 1) * QT],
                                lhsT=w_sb[d][:, 0:128], rhs=x_sb[d][:, cs],
                                start=(d == 0), stop=(d == DC - 1))
                for j2 in range(2):
                    js = slice(j2 * 1024, (j2 + 1) * 1024)
                    nc.vector.tensor_scalar_add(dst_a[:, js], pj[j2][:], b_a)

            def proj_passB(x_sb, w_sb, b_b, dst_b):
                # features 128:192 (head 2): emitted after attention has
                # started, so use the spare ps_c slot in 512-wide chunks
                for n4 in range(4):
                    cs = slice(n4 * QT, (n4 + 1) * QT)
                    pj = ps_c.tile([64, QT], f32, name="pj", tag="c")
                    for d in range(DC):
                        nc.tensor.matmul(pj[:], lhsT=w_sb[d][:, 128:GW],
                                         rhs=x_sb[d][:, cs],
                                         start=(d == 0), stop=(d == DC - 1))
                    nc.vector.tensor_scalar_add(dst_b[0:64, cs], pj[:], b_b)
                # mirror the 64-row b-half into partitions 64-127 so head-2
                # score matmuls can alternate PE row groups (pairing)
                nc.sync.dma_start(out=dst_b[64:128, :], in_=dst_b[0:64, :])

            proj_passA(xk_sb, wk_sb, bk_a, kT_a)
            proj_passA(xq_sb, wq_sb, bq_a, qT_a)

            # v projection (natural layout) + ones column per head.
            # Emitted per seq-tile, fused into q-tile 0's attention loop so
            # the ACT exp stream starts before v finishes projecting.
            v_sb = [None] * ST

            def v_proj(st):
                rs = slice(st * 128, (st + 1) * 128)
                pv = ps_o.tile([128, GW], f32, name="po", tag="po")
                for d in range(DC):
                    nc.tensor.matmul(pv[:], lhsT=xv_sb[d][:, rs],
                                     rhs=wv_sb[d][:],
                                     start=(d == 0), stop=(d == DC - 1))
                vt = cp.tile([128, HPG, D_K + 1], f16, name=f"vsb{st}")
                nc.vector.tensor_copy(out=vt[:, :, 0:D_K],
                                      in_=pv.rearrange("p (h w) -> p h w",
                                                       h=HPG))
                nc.vector.memset(vt[:, :, D_K:D_K + 1], 1.0)
                v_sb[st] = vt

            # ---- attention (transposed scores) + output projection ----
            # per-q-tile ctx tiles: a single [*, S] tile would make the
            # output projection of q-tile j falsely depend on q-tile j+1's
            # normalize writes (coarse tile deps)
            ctxT_a = [cp.tile([128, QT], f16, name=f"ctxTa{j}")
                      for j in range(NQT)]
            ctxT_b = [cp.tile([64, QT], f16, name=f"ctxTb{j}")
                      for j in range(NQT)]

            def head_slices(h, qt):
                if h == 0:
                    return kT_a[0:64], qT_a[0:64], ctxT_a[qt][0:64]
                if h == 1:
                    return kT_a[64:128], qT_a[64:128], ctxT_a[qt][64:128]
                return kT_b[0:64], qT_b[0:64], ctxT_b[qt][0:64]

            def normalize(C, h, qt):
                # ctxT = C[0:64] * (1/denom) + bv.  reciprocal_approx_fast
                # must read SBUF (garbage from PSUM on HW), so stage the
                # denominator row through SBUF first.
                _, _, ctx_dst = head_slices(h, qt)
                den = np_.tile([1, QT], f32, name="den")
                nc.vector.tensor_copy(out=den[:], in_=C[D_K:D_K + 1, :])
                r = np_.tile([1, QT], f32, name="r")
                nc.vector.reciprocal_approx_fast(out=r[:], in_=den[:])
                bc = np_.tile([128, QT], f32, name="bc")
                nc.gpsimd.partition_broadcast(bc[:], r[:])
                base = 64 if h == 1 else 0
                nc.vector.tensor_tensor(out=ctx_dst[:],
                                        in0=C[0:D_K, :],
                                        in1=bc[base:base + D_K, :],
                                        op=mult)
                nc.vector.tensor_scalar_add(ctx_dst[:], ctx_dst[:], bv_h[h])

            def attn_hp01(qt, fuse_v):
                # heads 0+1 interleaved: both go into one [128, 1024] PSUM
                # tile so exp runs as a single wide op, and the two score
                # matmuls (row groups 0-63 / 64-127) overlap on the PE.
                qs = slice(qt * QT, (qt + 1) * QT)
                Cs = {}
                for h in (0, 1):
                    Cs[h] = ps_c.tile([D_K + 1, QT], f32, name="C", tag="c")
                for kc in range(KC):
                    ks = slice(kc * 128, (kc + 1) * 128)
                    S2 = ps_s.tile([128, 2 * QT], f32, name="S", tag="s")
                    for h in (0, 1):
                        kT_h, qT_h, _ = head_slices(h, qt)
                        nc.tensor.matmul(S2[:, h * QT:(h + 1) * QT],
                                         lhsT=kT_h[:, ks], rhs=qT_h[:, qs])
                    e2 = ep.tile([128, 2 * QT], f16, name="expT")
                    nc.scalar.activation(e2[:], S2[:], Exp, scale=0.125)
                    if fuse_v:
                        v_proj(kc)
                    for h in (0, 1):
                        nc.tensor.matmul(Cs[h][:], lhsT=v_sb[kc][:, h, :],
                                         rhs=e2[:, h * QT:(h + 1) * QT],
                                         start=(kc == 0), stop=(kc == KC - 1))
                for h in (0, 1):
                    normalize(Cs[h], h, qt)

            def attn_h2(qt):
                # head 2: one [128, 1024] scores tile covers two k-chunks;
                # alternate PE row groups via the mirrored b-half
                qs = slice(qt * QT, (qt + 1) * QT)
                C2 = ps_c.tile([D_K + 1, QT], f32, name="C", tag="c")
                for kc2 in range(KC // 2):
                    S2 = ps_s.tile([128, 2 * QT], f32, name="S", tag="s")
                    for i in (0, 1):
                        kc = 2 * kc2 + i
                        rg = slice(64 * i, 64 * i + 64)
                        nc.tensor.matmul(
                            S2[:, i * QT:(i + 1) * QT],
                            lhsT=kT_b[rg, kc * 128:(kc + 1) * 128],
                            rhs=qT_b[rg, qs])
                    e2 = ep.tile([128, 2 * QT], f16, name="expT")
                    nc.scalar.activation(e2[:], S2[:], Exp, scale=0.125)
                    for i in (0, 1):
                        kc = 2 * kc2 + i
                        nc.tensor.matmul(C2[:], lhsT=v_sb[kc][:, 2, :],
                                         rhs=e2[:, i * QT:(i + 1) * QT],
                                         start=(kc == 0), stop=(kc == KC - 1))
                normalize(C2, 2, qt)



            def out_proj(qt, last=False):
                for st in range(QT // 128):
                    r0 = qt * QT + st * 128
                    ws = slice(st * 128, (st + 1) * 128)
                    osb = op.tile([128, D_MODEL], f32, name="osb")
                    for n, ns in enumerate((slice(0, 384), slice(384, 768))):
                        if last:
                            # attention is done: borrow the free S-pool
                            # slots so the tail pipelines
                            po = ps_s.tile([128, 384], f32, name="S",
                                           tag="s")
                        else:
                            po = ps_o.tile([128, 384], f32, name="po",
                                           tag="po")
                        nc.tensor.matmul(po[:], lhsT=ctxT_a[qt][:, ws],
                                         rhs=wo_a[:, ns],
                                         start=True, stop=False)
                        nc.tensor.matmul(po[:], lhsT=ctxT_b[qt][:, ws],
                                         rhs=wo_b[:, ns],
                                         start=False, stop=True)
                        nc.vector.tensor_copy(out=osb[:, ns], in_=po[:])
                    nc.sync.dma_start(out=out[r0:r0 + 128, :], in_=osb[:])

            # software pipeline: attention on heads 0+1 starts as soon as
            # the A-pass projections finish; the B-pass projections (head
            # 2's features), v-projection, and each q-tile's output
            # projection are emitted inside later ACT-bound attention
            # sections so the PE fills its slack instead of serializing.
            attn_hp01(0, fuse_v=True)
            proj_passB(xk_sb, wk_sb, bk_b, kT_b)
            proj_passB(xq_sb, wq_sb, bq_b, qT_b)
            attn_hp01(1, fuse_v=False)
            attn_h2(0)
            attn_h2(1)
            attn_hp01(2, fuse_v=False)
            out_proj(0)
            attn_h2(2)
            attn_hp01(3, fuse_v=False)
            out_proj(1)
            attn_h2(3)
            out_proj(2)
            out_proj(3, last=True)

    nc.compile()
    return nc


def _get_program():
    global _PROGRAM
    if _PROGRAM is None:
        _PROGRAM = _build_program()
    return _PROGRAM


def make_in_maps(query, key, value, Wq, bq, Wk, bk, Wv, bv, Wo, bo):
    """Build the 8 per-core input maps (host-side shard + transpose + cast)."""
    q32 = np.asarray(query, np.float32)
    k32 = np.asarray(key, np.float32)
    v32 = np.asarray(value, np.float32)
    xT = {}
    for b in range(B):
        xT[b] = (np.ascontiguousarray(q32[b].T).astype(np.float16),
                 np.ascontiguousarray(k32[b].T).astype(np.float16),
                 np.ascontiguousarray(v32[b].T).astype(np.float16))
    Wq = np.asarray(Wq, np.float32)
    Wk = np.asarray(Wk, np.float32)
    Wv = np.asarray(Wv, np.float32)
    Wo = np.asarray(Wo, np.float32)
    bq = np.asarray(bq, np.float32)
    bk = np.asarray(bk, np.float32)
    bv = np.asarray(bv, np.float32)
    in_maps = []
    for c in range(N_CORES):
        b, g = divmod(c, G)
        fs = slice(g * GW, (g + 1) * GW)
        xq, xk, xv = xT[b]
        # packed weights [128, WPK]: wq|wk|wv chunks (d-major), wo_a, wo_b
        wps = np.zeros((128, WPK), np.float16)
        for i, W in enumerate((Wk, Wq, Wv)):
            Ws = W[:, fs]
            for d in range(DC):
                wps[:, (i * DC + d) * GW:(i * DC + d + 1) * GW] = \
                    Ws[d * 128:(d + 1) * 128, :].astype(np.float16)
        Wos = Wo[fs, :]
        wps[:, 3 * DC * GW:3 * DC * GW + D_MODEL] = \
            Wos[0:128, :].astype(np.float16)
        wps[0:64, 3 * DC * GW + D_MODEL:WPK] = \
            Wos[128:GW, :].astype(np.float16)
        # packed biases [128, 8] f32
        bps = np.zeros((128, BPK), np.float32)
        bps[:, 0] = bq[fs][0:128]
        bps[0:64, 1] = bq[fs][128:GW]
        bps[:, 2] = bk[fs][0:128]
        bps[0:64, 3] = bk[fs][128:GW]
        for h in range(HPG):
            bps[0:64, 4 + h] = bv[fs][h * 64:(h + 1) * 64]
        in_maps.append({
            "xqT": xq, "xkT": xk, "xvT": xv,
            "wpk": wps, "bpk": bps,
        })
    return in_maps


def combine_outputs(results, bo):
    """Sum the per-core partial outputs into the full [B, S, D] output."""
    bo = np.asarray(bo, np.float32)
    out = np.zeros((B, S, D_MODEL), np.float32)
    for c in range(N_CORES):
        b = c // G
        out[b] += np.asarray(results[c]["out"], np.float32)
    out += bo[None, None, :]
    return out


def kernel(**inputs):
    from concourse.bass_utils import run_bass_kernel_spmd

    nc = _get_program()
    in_maps = make_in_maps(**inputs)
    res = run_bass_kernel_spmd(nc, in_maps, list(range(N_CORES)))
    return combine_outputs(res.results, inputs["bo"])


# revision 28
# speedup vs baseline: 1.0006x; 1.0006x over previous
"""Multi-head attention (B=2, S=2048, D=768, H=12) on 8 trn2 NeuronCores.

Sharding: batch x head-group data/tensor parallel. Core c = b*4+g handles
batch b and heads [3g, 3g+3) (a 192-wide slice of the QKV projections and
the matching 192-row slice of Wo). Each core emits a partial [2048, 768]
output; the host sums the 4 head-group partials per batch and adds bo.

Device layout notes:
- Inputs are transposed on host to [d_model, seq] and cast to fp16 so the
  TensorEngine (which contracts over the partition dim) can consume them
  directly; all matmuls run on fp16 operands with fp32 PSUM accumulation.
- Attention works on transposed scores sT[k, q] so softmax's sum over k
  becomes a matmul reduction: v is augmented with a ones column, so the
  ctx matmul yields both ctx^T and the softmax denominator in one pass.
  exp() needs no max-subtraction: |scores/8| <= ~11 for this problem.
- Normalization multiplies ctx^T by 1/denom broadcast across partitions
  (GPSIMD partition_broadcast), then the output projection runs from
  ctx^T directly.
- Heads 0/1 live at SBUF partitions 0-63/64-127 so their score matmuls
  land in different PE row groups and overlap; head 2's operands are
  mirrored into both halves for the same reason.
- The output projection for q-tile j is emitted after q-tile j+1's
  attention so the PE stream never stalls on the normalize chain.
"""

import numpy as np

D_MODEL = 768
NUM_HEADS = 12
D_K = 64
B = 2
S = 2048
N_CORES = 8
G = 4              # head groups (cores per batch)
GW = D_MODEL // G  # 192 features per group = 3 heads
HPG = 3            # heads per group
DC = D_MODEL // 128  # 6 d_model chunks
QT = 512           # q-tile width
NQT = S // QT      # 4
KC = S // 128      # 16 k chunks
ST = S // 128      # 16 seq tiles
WPK = 3 * DC * GW + 2 * D_MODEL  # packed weights columns: 4992
BPK = 8            # packed bias columns

_PROGRAM = None


def _build_program():
    from concourse import bacc, tile
    import concourse.mybir as mybir

    f16 = mybir.dt.float16
    f32 = mybir.dt.float32
    Exp = mybir.ActivationFunctionType.Exp
    mult = mybir.AluOpType.mult

    nc = bacc.Bacc("TRN2", target_bir_lowering=False, debug=False,
                   enable_asserts=False)

    xqT = nc.dram_tensor("xqT", [D_MODEL, S], f16, kind="ExternalInput")
    xkT = nc.dram_tensor("xkT", [D_MODEL, S], f16, kind="ExternalInput")
    xvT = nc.dram_tensor("xvT", [D_MODEL, S], f16, kind="ExternalInput")
    wpk = nc.dram_tensor("wpk", [128, WPK], f16, kind="ExternalInput")
    bpk = nc.dram_tensor("bpk", [128, BPK], f32, kind="ExternalInput")
    out = nc.dram_tensor("out", [S, D_MODEL], f32, kind="ExternalOutput")

    with tile.TileContext(nc) as tc:
        with tc.tile_pool(name="const", bufs=1) as cp, \
             tc.tile_pool(name="expp", bufs=6) as ep, \
             tc.tile_pool(name="normp", bufs=2) as np_, \
             tc.tile_pool(name="outp", bufs=2) as op, \
             tc.tile_pool(name="ps_s", bufs=2, space="PSUM") as ps_s, \
             tc.tile_pool(name="ps_c", bufs=3, space="PSUM") as ps_c, \
             tc.tile_pool(name="ps_o", bufs=1, space="PSUM") as ps_o:

            # ---- packed weights + biases. wk occupies the first
            # columns and ships in its own DMA so the k-projection can
            # start as soon as possible ----
            wps = cp.tile([128, WPK], f16, name="wps")
            nc.sync.dma_start(out=wps[:, 0:DC * GW], in_=wpk[:, 0:DC * GW])
            bps = cp.tile([128, BPK], f32, name="bps")
            nc.sync.dma_start(out=bps[:], in_=bpk[:])
            wk_sb = [wps[:, d * GW:(d + 1) * GW] for d in range(DC)]
            wq_sb = [wps[:, DC * GW + d * GW:DC * GW + (d + 1) * GW]
                     for d in range(DC)]
            wv_sb = [wps[:, 2 * DC * GW + d * GW:2 * DC * GW + (d + 1) * GW]
                     for d in range(DC)]
            wo_a = wps[:, 3 * DC * GW:3 * DC * GW + D_MODEL]
            wo_b = wps[0:64, 3 * DC * GW + D_MODEL:WPK]
            bq_a, bq_b = bps[:, 0:1], bps[0:64, 1:2]
            bk_a, bk_b = bps[:, 2:3], bps[0:64, 3:4]
            bv_h = [bps[0:64, 4 + h:5 + h] for h in range(HPG)]

            # ---- inputs: k first, then v, then q (attention needs full
            # kT and v before it can start, q only per-tile) ----
            xq_sb, xk_sb, xv_sb = [], [], []
            for d in range(DC):
                t = cp.tile([128, S], f16, name=f"xk{d}")
                nc.sync.dma_start(out=t[:], in_=xkT[d * 128:(d + 1) * 128, :])
                xk_sb.append(t)
            # remaining weights (wq/wv/wo) ship after xk so the k
            # projection's data isn't stuck behind them in the DMA queue
            nc.sync.dma_start(out=wps[:, DC * GW:WPK],
                              in_=wpk[:, DC * GW:WPK])
            for d in range(DC):
                t = cp.tile([128, S], f16, name=f"xq{d}")
                nc.sync.dma_start(out=t[:], in_=xqT[d * 128:(d + 1) * 128, :])
                xq_sb.append(t)
            for d in range(DC):
                t = cp.tile([128, S], f16, name=f"xv{d}")
                nc.sync.dma_start(out=t[:], in_=xvT[d * 128:(d + 1) * 128, :])
                xv_sb.append(t)

            # ---- projections. Order: kT, v, qT (dependency order) ----
            qT_a = cp.tile([128, S], f16, name="qT_a")
            qT_b = cp.tile([128, S], f16, name="qT_b")
            kT_a = cp.tile([128, S], f16, name="kT_a")
            kT_b = cp.tile([128, S], f16, name="kT_b")

            def proj_passA(x_sb, w_sb, b_a, dst_a):
                # features 0:128 (heads 0+1), d-outer accumulation: each
                # input chunk is consumed as it arrives from HBM. The
                # attention S-pool is idle here, so borrow its slots.
                pj = [ps_s.tile([128, 2 * QT], f32, name="S", tag="s")
                      for _ in range(2)]
                for d in range(DC):
                    for j2 in range(2):
                        for n in range(2):
                            cs = slice(j2 * 1024 + n * QT,
                                       j2 * 1024 + (n + 1) * QT)
                            nc.tensor.matmul(
                                pj[j2][:, n * QT:(n + 1) * QT],
                                lhsT=w_sb[d][:, 0:128], rhs=x_sb[d][:, cs],
                                start=(d == 0), stop=(d == DC - 1))
                for j2 in range(2):
                    js = slice(j2 * 1024, (j2 + 1) * 1024)
                    nc.vector.tensor_scalar_add(dst_a[:, js], pj[j2][:], b_a)

            def proj_passB(x_sb, w_sb, b_b, dst_b):
                # features 128:192 (head 2): emitted after attention has
                # started, so use the spare ps_c slot in 512-wide chunks
                for n4 in range(4):
                    cs = slice(n4 * QT, (n4 + 1) * QT)
                    pj = ps_c.tile([64, QT], f32, name="pj", tag="c")
                    for d in range(DC):
                        nc.tensor.matmul(pj[:], lhsT=w_sb[d][:, 128:GW],
                                         rhs=x_sb[d][:, cs],
                                         start=(d == 0), stop=(d == DC - 1))
                    nc.vector.tensor_scalar_add(dst_b[0:64, cs], pj[:], b_b)
                # mirror the 64-row b-half into partitions 64-127 so head-2
                # score matmuls can alternate PE row groups (pairing)
                nc.sync.dma_start(out=dst_b[64:128, :], in_=dst_b[0:64, :])

            proj_passA(xk_sb, wk_sb, bk_a, kT_a)
            proj_passA(xq_sb, wq_sb, bq_a, qT_a)

            # v projection (natural layout) + ones column per head.
            # Emitted per seq-tile, fused into q-tile 0's attention loop so
            # the ACT exp stream starts before v finishes projecting.
            v_sb = [None] * ST

            def v_proj(st):
                rs = slice(st * 128, (st + 1) * 128)
                pv = ps_o.tile([128, GW], f32, name="po", tag="po")
                for d in range(DC):
                    nc.tensor.matmul(pv[:], lhsT=xv_sb[d][:, rs],
                                     rhs=wv_sb[d][:],
                                     start=(d == 0), stop=(d == DC - 1))
                vt = cp.tile([128, HPG, D_K + 1], f16, name=f"vsb{st}")
                nc.vector.tensor_copy(out=vt[:, :, 0:D_K],
                                      in_=pv.rearrange("p (h w) -> p h w",
                                                       h=HPG))
                nc.vector.memset(vt[:, :, D_K:D_K + 1], 1.0)
                v_sb[st] = vt

            # ---- attention (transposed scores) + output projection ----
            # per-q-tile ctx tiles: a single [*, S] tile would make the
            # output projection of q-tile j falsely depend on q-tile j+1's
            # normalize writes (coarse tile deps)
            ctxT_a = [cp.tile([128, QT], f16, name=f"ctxTa{j}")
                      for j in range(NQT)]
            ctxT_b = [cp.tile([64, QT], f16, name=f"ctxTb{j}")
                      for j in range(NQT)]

            def head_slices(h, qt):
                if h == 0:
                    return kT_a[0:64], qT_a[0:64], ctxT_a[qt][0:64]
                if h == 1:
                    return kT_a[64:128], qT_a[64:128], ctxT_a[qt][64:128]
                return kT_b[0:64], qT_b[0:64], ctxT_b[qt][0:64]

            def normalize(C, h, qt):
                # ctxT = C[0:64] * (1/denom) + bv.  reciprocal_approx_fast
                # must read SBUF (garbage from PSUM on HW), so stage the
                # denominator row through SBUF first.
                _, _, ctx_dst = head_slices(h, qt)
                den = np_.tile([1, QT], f32, name="den")
                nc.vector.tensor_copy(out=den[:], in_=C[D_K:D_K + 1, :])
                r = np_.tile([1, QT], f32, name="r")
                nc.vector.reciprocal_approx_fast(out=r[:], in_=den[:])
                bc = np_.tile([128, QT], f32, name="bc")
                nc.gpsimd.partition_broadcast(bc[:], r[:])
                base = 64 if h == 1 else 0
                nc.vector.tensor_tensor(out=ctx_dst[:],
                                        in0=C[0:D_K, :],
                                        in1=bc[base:base + D_K, :],
                                        op=mult)
                nc.vector.tensor_scalar_add(ctx_dst[:], ctx_dst[:], bv_h[h])

            def attn_hp01(qt, fuse_v):
                # heads 0+1 interleaved: both go into one [128, 1024] PSUM
                # tile so exp runs as a single wide op, and the two score
                # matmuls (row groups 0-63 / 64-127) overlap on the PE.
                qs = slice(qt * QT, (qt + 1) * QT)
                Cs = {}
                for h in (0, 1):
                    Cs[h] = ps_c.tile([D_K + 1, QT], f32, name="C", tag="c")
                for kc in range(KC):
                    ks = slice(kc * 128, (kc + 1) * 128)
                    S2 = ps_s.tile([128, 2 * QT], f32, name="S", tag="s")
                    for h in (0, 1):
                        kT_h, qT_h, _ = head_slices(h, qt)
                        nc.tensor.matmul(S2[:, h * QT:(h + 1) * QT],
                                         lhsT=kT_h[:, ks], rhs=qT_h[:, qs])
                    e2 = ep.tile([128, 2 * QT], f16, name="expT")
                    nc.scalar.activation(e2[:], S2[:], Exp, scale=0.125)
                    if fuse_v:
                        v_proj(kc)
                    for h in (0, 1):
                        nc.tensor.matmul(Cs[h][:], lhsT=v_sb[kc][:, h, :],
                                         rhs=e2[:, h * QT:(h + 1) * QT],
                                         start=(kc == 0), stop=(kc == KC - 1))
                for h in (0, 1):
                    normalize(Cs[h], h, qt)

            def attn_h2(qt):
                # head 2: one [128, 1024] scores tile covers two k-chunks;
                # alternate PE row groups via the mirrored b-half
                qs = slice(qt * QT, (qt + 1) * QT)
                C2 = ps_c.tile([D_K + 1, QT], f32, name="C", tag="c")
                for kc2 in range(KC // 2):
                    S2 = ps_s.tile([128, 2 * QT], f32, name="S", tag="s")
                    for i in (0, 1):
                        kc = 2 * kc2 + i
                        rg = slice(64 * i, 64 * i + 64)
                        nc.tensor.matmul(
                            S2[:, i * QT:(i + 1) * QT],
                            lhsT=kT_b[rg, kc * 128:(kc + 1) * 128],
                            rhs=qT_b[rg, qs])
                    e2 = ep.tile([128, 2 * QT], f16, name="expT")
                    nc.scalar.activation(e2[:], S2[:], Exp, scale=0.125)
                    for i in (0, 1):
                        kc = 2 * kc2 + i
                        nc.tensor.matmul(C2[:], lhsT=v_sb[kc][:, 2, :],
                                         rhs=e2[:, i * QT:(i + 1) * QT],
                                         start=(kc == 0), stop=(kc == KC - 1))
                normalize(C2, 2, qt)



            def out_proj(qt, last=False):
                for st in range(QT // 128):
                    r0 = qt * QT + st * 128
                    ws = slice(st * 128, (st + 1) * 128)
                    osb = op.tile([128, D_MODEL], f32, name="osb")
                    for n, ns in enumerate((slice(0, 384), slice(384, 768))):
                        if last:
                            # attention is done: borrow the free S-pool
                            # slots so the tail pipelines
                            po = ps_s.tile([128, 384], f32, name="S",
                                           tag="s")
                        else:
                            po = ps_o.tile([128, 384], f32, name="po",
                                           tag="po")
                        nc.tensor.matmul(po[:], lhsT=ctxT_a[qt][:, ws],
                                         rhs=wo_a[:, ns],
                                         start=True, stop=False)
                        nc.tensor.matmul(po[:], lhsT=ctxT_b[qt][:, ws],
                                         rhs=wo_b[:, ns],
                                         start=False, stop=True)
                        nc.vector.tensor_copy(out=osb[:, ns], in_=po[:])
                    nc.sync.dma_start(out=out[r0:r0 + 128, :], in_=osb[:])

            # software pipeline: attention on heads 0+1 starts as soon as
            # the A-pass projections finish; the B-pass projections (head
            # 2's features), v-projection, and each q-tile's output
            # projection are emitted inside later ACT-bound attention
            # sections so the PE fills its slack instead of serializing.
            attn_hp01(0, fuse_v=True)
            proj_passB(xk_sb, wk_sb, bk_b, kT_b)
            proj_passB(xq_sb, wq_sb, bq_b, qT_b)
            attn_hp01(1, fuse_v=False)
            attn_hp01(2, fuse_v=False)
            attn_hp01(3, fuse_v=False)
            attn_h2(0)
            out_proj(0)
            attn_h2(1)
            out_proj(1)
            attn_h2(2)
            out_proj(2)
            attn_h2(3)
            out_proj(3, last=True)

    nc.compile()
    return nc


def _get_program():
    global _PROGRAM
    if _PROGRAM is None:
        _PROGRAM = _build_program()
    return _PROGRAM


def make_in_maps(query, key, value, Wq, bq, Wk, bk, Wv, bv, Wo, bo):
    """Build the 8 per-core input maps (host-side shard + transpose + cast)."""
    q32 = np.asarray(query, np.float32)
    k32 = np.asarray(key, np.float32)
    v32 = np.asarray(value, np.float32)
    xT = {}
    for b in range(B):
        xT[b] = (np.ascontiguousarray(q32[b].T).astype(np.float16),
                 np.ascontiguousarray(k32[b].T).astype(np.float16),
                 np.ascontiguousarray(v32[b].T).astype(np.float16))
    Wq = np.asarray(Wq, np.float32)
    Wk = np.asarray(Wk, np.float32)
    Wv = np.asarray(Wv, np.float32)
    Wo = np.asarray(Wo, np.float32)
    bq = np.asarray(bq, np.float32)
    bk = np.asarray(bk, np.float32)
    bv = np.asarray(bv, np.float32)
    in_maps = []
    for c in range(N_CORES):
        b, g = divmod(c, G)
        fs = slice(g * GW, (g + 1) * GW)
        xq, xk, xv = xT[b]
        # packed weights [128, WPK]: wq|wk|wv chunks (d-major), wo_a, wo_b
        wps = np.zeros((128, WPK), np.float16)
        for i, W in enumerate((Wk, Wq, Wv)):
            Ws = W[:, fs]
            for d in range(DC):
                wps[:, (i * DC + d) * GW:(i * DC + d + 1) * GW] = \
                    Ws[d * 128:(d + 1) * 128, :].astype(np.float16)
        Wos = Wo[fs, :]
        wps[:, 3 * DC * GW:3 * DC * GW + D_MODEL] = \
            Wos[0:128, :].astype(np.float16)
        wps[0:64, 3 * DC * GW + D_MODEL:WPK] = \
            Wos[128:GW, :].astype(np.float16)
        # packed biases [128, 8] f32
        bps = np.zeros((128, BPK), np.float32)
        bps[:, 0] = bq[fs][0:128]
        bps[0:64, 1] = bq[fs][128:GW]
        bps[:, 2] = bk[fs][0:128]
        bps[0:64, 3] = bk[fs][128:GW]
        for h in range(HPG):
            bps[0:64, 4 + h] = bv[fs][h * 64:(h + 1) * 64]
        in_maps.append({
            "xqT": xq, "xkT": xk, "xvT": xv,
            "wpk": wps, "bpk": bps,
        })
    return in_maps


def combine_outputs(results, bo):
    """Sum the per-core partial outputs into the full [B, S, D] output."""
    bo = np.asarray(bo, np.float32)
    out = np.zeros((B, S, D_MODEL), np.float32)
    for c in range(N_CORES):
        b = c // G
        out[b] += np.asarray(results[c]["out"], np.float32)
    out += bo[None, None, :]
    return out


def kernel(**inputs):
    from concourse.bass_utils import run_bass_kernel_spmd

    nc = _get_program()
    in_maps = make_in_maps(**inputs)
    res = run_bass_kernel_spmd(nc, in_maps, list(range(N_CORES)))
    return combine_outputs(res.results, inputs["bo"])
